# revision 1
# baseline (speedup 1.0000x reference)
"""Trainium2 Bass kernel for nn_Cifar10_JointMembership (v2: scatter-gather).

Math (closed form of the reference 2-qubit circuit):
  a = x[b, i0], bb = x[b, i1]  (gathered pixel pairs, full angles)
  out[b, 2p,   c] = 0.5 + A_c*cos(a) + B_c*sin(a)*sin(bb),
      A_c = 0.5*cos(theta_c), B_c = -0.5*sin(theta_c)
  out[b, 2p+1, c] = 0.5 + 0.5*cos(a)*cos(bb)   (same for all c)

Sharding: pure data parallel, 128 batch rows per NeuronCore (batch dim of
x and pair_idx split across the 8 cores); theta replicated.

Gather strategy: the stock GPSIMD IndirectCopy gather costs ~25ns per
16-partition index column and needs 16x columns for per-partition indices
(~377us for 920 idx/row — it was 96% of the previous kernel's runtime).
Instead the gather is INVERTED on the host into local_scatter maps
(per-partition independent indices): one scatter streams each 3072-pixel
row once and drops every pixel into the dest slot of its first use; T tail
scatters stream the 920-slot val buffer and copy first-use slots into the
dest slots of repeated uses (duplicate pixel draws); the disjoint writes
are then summed.  Host-side work is index-layout preprocessing only (the
inverse maps); all data movement/compute stays on the device.

Range reduction: ACT Sin is accurate only on [-pi, pi] (HW-measured), so
sin/cos arguments are wrapped with the fused add_range_wrap DVE op
(shift pi/2 for cos).  Compute fp16 downstream of the gather; output fp16,
cast to f32 on the host (harness gate is rel_err < 2e-2; this lands ~2.5e-4).
"""
import numpy as np

import concourse.bass as bass
import concourse.mybir as mybir
from concourse.tile import TileContext
from concourse import library_config

N_CORES = 8
B_FULL = 1024
B = B_FULL // N_CORES
NPIX = 3072
NPAIR = 460
NIDX = 2 * NPAIR
NCLS = 10
F32 = mybir.dt.float32
F16 = mybir.dt.float16
I16 = mybir.dt.int16
ALU = mybir.AluOpType
PI = float(np.pi)
HALF_PI = float(np.pi / 2)
TWO_PI = float(2 * np.pi)


def _legalize_sync_waits(nc):
    """Cap sync-waits per instruction (this walrus allows 1, or 2 on
    EventSemaphore); hoist the excess onto EventSemaphore instructions."""
    n_new = 0
    for f in nc.m.functions:
        for bb in f.blocks:
            out = []
            for inst in bb.instructions:
                si = inst.sync_info
                waits = list(si.on_wait) if si is not None and si.on_wait else []
                cap = 2 if inst.opcode == "EventSemaphore" else 1
                if len(waits) > cap:
                    keep, hoist = waits[:cap], waits[cap:]
                    del si.on_wait[:]
                    for w in keep:
                        si.on_wait.append(w)
                    while hoist:
                        chunk, hoist = hoist[:2], hoist[2:]
                        n_new += 1
                        ev = mybir.InstEventSemaphore(
                            name=f"{inst.name}-hw{n_new}",
                            ins=[],
                            outs=[],
                            engine=inst.engine,
                            sync_info=mybir.SyncInfo(on_wait=chunk, on_update=[]),
                        )
                        out.append(ev)
                out.append(inst)
            bb.instructions = out
    return nc


def build_kernel(R, n_repeat=1, n_chunks=4, sin_direct=False, span_chunks=None):
    """One NeuronCore's program: 128 batch rows, R = 1 + tail rounds."""
    Sin = mybir.ActivationFunctionType.Sin
    Copy = mybir.ActivationFunctionType.Copy
    out_dt = F16

    T = R - 1
    nc = bass.Bass(detect_race_conditions=False)
    xd = nc.dram_tensor("x16", [B, NPIX], F16, kind="ExternalInput")
    idxd = nc.dram_tensor("sidx", [B, NPIX], I16, kind="ExternalInput")
    if T > 0:
        tld = nc.dram_tensor("tidx", [B, T * NIDX], I16, kind="ExternalInput")
    td = nc.dram_tensor("theta", [1, NCLS], F32, kind="ExternalInput")
    od = nc.dram_tensor("out", [B, NIDX * NCLS], out_dt, kind="ExternalOutput")

    assert NPAIR % n_chunks == 0
    PCH = NPAIR // n_chunks
    if span_chunks is None:
        span_chunks = (1,) * n_chunks
    assert sum(span_chunks) == n_chunks

    with TileContext(nc) as tc:
        with (
            tc.tile_pool(name="const", bufs=1) as cpool,
            tc.tile_pool(name="inp", bufs=1) as ipool,
            tc.tile_pool(name="val", bufs=2) as vpool,
            tc.tile_pool(name="mid", bufs=3) as mpool,
            tc.tile_pool(name="outp", bufs=2) as opool,
            tc.tile_pool(name="tcc", bufs=4) as tccpool,
        ):
            nc.gpsimd.load_library(library_config.local_scatter)

            # ---- constants / theta coefficients ----
            zbias = cpool.tile([B, 1], F32, tag="zbias")
            nc.vector.memset(zbias[:], 0.0)
            halfpi = cpool.tile([B, 1], F32, tag="halfpi")
            nc.vector.memset(halfpi[:], HALF_PI)

            th = cpool.tile([B, NCLS], F32, tag="th")
            nc.scalar.dma_start(out=th[:], in_=td[:].to_broadcast((B, NCLS)))
            A = cpool.tile([B, NCLS], F32, tag="A")
            Bc = cpool.tile([B, NCLS], F32, tag="Bc")
            if sin_direct:
                nc.scalar.activation(A[:], th[:], Sin, bias=halfpi[:, 0:1], scale=-1.0)
                nc.scalar.activation(Bc[:], th[:], Sin, bias=zbias[:, 0:1])
            else:
                thw = cpool.tile([B, NCLS], F32, tag="thw")
                thc = cpool.tile([B, NCLS], F32, tag="thc")
                nc.vector.add_range_wrap(thw[:], th[:], 0.0, PI, TWO_PI)
                nc.vector.add_range_wrap(thc[:], th[:], HALF_PI, PI, TWO_PI)
                nc.scalar.activation(Bc[:], thw[:], Sin, bias=zbias[:, 0:1])
                nc.scalar.activation(A[:], thc[:], Sin, bias=zbias[:, 0:1])
            nc.vector.tensor_scalar_mul(A[:], A[:], 0.5)
            nc.vector.tensor_scalar_mul(Bc[:], Bc[:], -0.5)

            # ---- inputs ----
            xt = ipool.tile([B, NPIX], F16, tag="xt")
            nc.sync.dma_start(out=xt[:], in_=xd[:])
            sidx = ipool.tile([B, NPIX], I16, tag="sidx")
            nc.sync.dma_start(out=sidx[:], in_=idxd[:])
            tidxs = []
            for t in range(T):
                ti = ipool.tile([B, NIDX], I16, tag=f"tidx{t}")
                nc.scalar.dma_start(
                    out=ti[:], in_=tld[:, t * NIDX : (t + 1) * NIDX]
                )
                tidxs.append(ti)

            for rep in range(n_repeat):
                # ---- gather: first-use scatter from x, tail scatters from
                # val (duplicate slots copy their pixel's first-use slot),
                # then sum the disjoint writes ----
                v0 = vpool.tile([B, NIDX], F16, tag="v0")
                nc.gpsimd.local_scatter(
                    v0[:], xt[:], sidx[:],
                    channels=128, num_elems=NIDX, num_idxs=NPIX,
                )
                dsts = []
                for t in range(T):
                    dst = vpool.tile([B, NIDX], F16, tag=f"dst{t}")
                    nc.gpsimd.local_scatter(
                        dst[:], v0[:], tidxs[t][:],
                        channels=128, num_elems=NIDX, num_idxs=NIDX,
                    )
                    dsts.append(dst)
                # tree-sum the disjoint writes so partial adds overlap the
                # still-running tail scatters
                level = [v0] + dsts
                li = 0
                while len(level) > 1:
                    nxt = []
                    for i in range(0, len(level) - 1, 2):
                        s = vpool.tile([B, NIDX], F16, tag=f"sum{li}_{i}")
                        nc.vector.tensor_add(s[:], level[i][:], level[i + 1][:])
                        nxt.append(s)
                    if len(level) % 2:
                        nxt.append(level[-1])
                    level = nxt
                    li += 1
                val = level[0]
                # val[:, 0:460] = a, val[:, 460:920] = b

                ca = mpool.tile([B, NPAIR], F16, tag="ca")
                sa = mpool.tile([B, NPAIR], F16, tag="sa")
                cb = mpool.tile([B, NPAIR], F16, tag="cb")
                sb = mpool.tile([B, NPAIR], F16, tag="sb")

                def trig(k):
                    sl = slice(k * PCH, (k + 1) * PCH)
                    av = val[:, k * PCH : (k + 1) * PCH]
                    bv = val[:, NPAIR + k * PCH : NPAIR + (k + 1) * PCH]
                    if sin_direct:
                        nc.scalar.activation(sa[:, sl], av, Sin, bias=zbias[:, 0:1])
                        nc.scalar.activation(
                            ca[:, sl], av, Sin, bias=halfpi[:, 0:1], scale=-1.0
                        )
                        nc.scalar.activation(sb[:, sl], bv, Sin, bias=zbias[:, 0:1])
                        nc.scalar.activation(
                            cb[:, sl], bv, Sin, bias=halfpi[:, 0:1], scale=-1.0
                        )
                    else:
                        aw = mpool.tile([B, PCH], F32, tag="aw")
                        ac = mpool.tile([B, PCH], F32, tag="ac")
                        bw = mpool.tile([B, PCH], F32, tag="bw")
                        bc = mpool.tile([B, PCH], F32, tag="bc")
                        nc.vector.add_range_wrap(aw[:], av, 0.0, PI, TWO_PI)
                        nc.vector.add_range_wrap(ac[:], av, HALF_PI, PI, TWO_PI)
                        nc.vector.add_range_wrap(bw[:], bv, 0.0, PI, TWO_PI)
                        nc.vector.add_range_wrap(bc[:], bv, HALF_PI, PI, TWO_PI)
                        nc.scalar.activation(sa[:, sl], aw[:], Sin, bias=zbias[:, 0:1])
                        nc.scalar.activation(ca[:, sl], ac[:], Sin, bias=zbias[:, 0:1])
                        nc.scalar.activation(sb[:, sl], bw[:], Sin, bias=zbias[:, 0:1])
                        nc.scalar.activation(cb[:, sl], bc[:], Sin, bias=zbias[:, 0:1])

                def class_span(p0, PS):
                    sl = slice(p0, p0 + PS)
                    v = mpool.tile([B, PS], F16, tag="v")
                    wv = mpool.tile([B, PS], F16, tag="wv")
                    nc.vector.tensor_mul(v[:], sa[:, sl], sb[:, sl])
                    nc.vector.tensor_mul(wv[:], ca[:, sl], cb[:, sl])

                    ob = opool.tile([B, PS * 2 * NCLS], out_dt, tag="ob")
                    ob3 = ob[:].rearrange("p (t k) -> p t k", k=2 * NCLS)

                    om = mpool.tile([B, PS], out_dt, tag="om")
                    nc.scalar.activation(om[:], wv[:], Copy, bias=0.5, scale=0.5)
                    nc.scalar.activation(
                        ob3[:, :, NCLS : 2 * NCLS],
                        om[:, :, None].broadcast_to((B, PS, NCLS)),
                        Copy,
                    )
                    for c in range(NCLS):
                        tcc = tccpool.tile([B, PS], F32, tag="tcc")
                        nc.vector.tensor_scalar(
                            tcc[:], v[:], Bc[:, c : c + 1], 0.5, ALU.mult, ALU.add
                        )
                        nc.vector.scalar_tensor_tensor(
                            ob[:, c : PS * 2 * NCLS : 2 * NCLS],
                            ca[:, sl],
                            A[:, c : c + 1],
                            tcc[:],
                            ALU.mult,
                            ALU.add,
                        )
                    nc.sync.dma_start(
                        out=od[:, p0 * 2 * NCLS : (p0 + PS) * 2 * NCLS],
                        in_=ob[:],
                    )

                for k in range(n_chunks):
                    trig(k)
                    done = k + 1
                    acc_ch = 0
                    for nch in span_chunks:
                        if acc_ch + nch == done:
                            class_span(acc_ch * PCH, nch * PCH)
                            break
                        acc_ch += nch

    mybir.codegen_inst_isa_subclasses(nc)
    return _legalize_sync_waits(nc)


def _scatter_maps(pidx):
    """Invert per-row gather lists into cascade scatter maps.
    pidx: [P, 920] pixel indices, pair-interleaved. Dest slots are
    deinterleaved: even j -> j//2 (a-block), odd j -> 460 + j//2.
    Returns (m1 [P, NPIX] int16: pixel -> first-use dest or -1,
             tails [T, P, NIDX] int16: first-use dest -> (t+1)-th-use dest)."""
    P, J = pidx.shape
    jj = np.arange(J)
    dest_j = np.where(jj % 2 == 0, jj // 2, NPAIR + jj // 2).astype(np.int16)
    p_arr = np.repeat(np.arange(P), J)
    j_arr = np.tile(jj, P)
    pix = pidx.reshape(-1).astype(np.int64)
    order = np.lexsort((j_arr, pix, p_arr))
    ps, xs = p_arr[order], pix[order]
    js = dest_j[j_arr[order]]
    newgrp = np.r_[True, (ps[1:] != ps[:-1]) | (xs[1:] != xs[:-1])]
    pos = np.arange(P * J)
    first_pos = np.maximum.accumulate(np.where(newgrp, pos, 0))
    occ = pos - first_pos
    first_js = js[first_pos]
    T = int(occ.max())
    m1 = np.full((P, NPIX), -1, np.int16)
    sel = occ == 0
    m1[ps[sel], xs[sel]] = js[sel]
    tails = np.full((T, P, NIDX), -1, np.int16)
    sel = occ >= 1
    tails[occ[sel] - 1, ps[sel], first_js[sel]] = js[sel]
    return m1, tails


def _prep_inputs(x, theta, pair_idx, T_pad=None):
    """Full inputs -> per-core input maps. Host work: shard, dtype-narrow,
    and index-layout inversion (scatter maps)."""
    x = np.asarray(x, dtype=np.float32).reshape(B_FULL, NPIX)
    theta = np.ascontiguousarray(np.asarray(theta, dtype=np.float32).reshape(1, NCLS))
    pidx = np.asarray(pair_idx).reshape(B_FULL, NIDX).astype(np.int64)
    maps = []
    for k in range(N_CORES):
        sl = slice(k * B, (k + 1) * B)
        maps.append(_scatter_maps(pidx[sl]))
    T = max(m[1].shape[0] for m in maps)
    if T_pad is not None:
        T = max(T, T_pad)
    in_maps = []
    for k in range(N_CORES):
        sl = slice(k * B, (k + 1) * B)
        m1, tails = maps[k]
        if tails.shape[0] < T:
            pad = np.full((T - tails.shape[0], B, NIDX), -1, np.int16)
            tails = np.concatenate([tails, pad], axis=0)
        in_maps.append(
            {
                "x16": np.ascontiguousarray(x[sl].astype(np.float16)),
                "sidx": np.ascontiguousarray(m1),
                "tidx": np.ascontiguousarray(
                    tails.transpose(1, 0, 2).reshape(B, T * NIDX)
                ),
                "theta": theta,
            }
        )
    return in_maps, T + 1


_CACHED = {}


def kernel(x, theta, pair_idx):
    from concourse.bass_utils import run_bass_kernel_spmd

    in_maps, R = _prep_inputs(x, theta, pair_idx)
    if R not in _CACHED:
        _CACHED[R] = build_kernel(R)
    nc = _CACHED[R]
    res = run_bass_kernel_spmd(nc, in_maps, core_ids=list(range(N_CORES)))
    out = np.concatenate([r["out"] for r in res.results], axis=0)
    return out.astype(np.float32).reshape(B_FULL, NIDX, NCLS)

